# revision 28
# baseline (speedup 1.0000x reference)
# GNN edge-apply MLP kernel for Trainium2 (Bass/Tile), 8-core SPMD.
#
# reference semantics:
#   feat = concat(node_feats[src], node_feats[dst], axis=1)      # [E, 2048]
#   h    = relu(feat @ W1 + b1)                                  # [E, 1024]
#   out  = h @ W2 + b2                                           # [E, 1024]
#
# Sharding: edges split evenly across 8 cores (8192 each); node table and
# weights replicated.  Edge e of a core shard maps to (p, t) = (e // T,
# e % T): index loads and output stores are contiguous per partition.
#
# v7 design.  The f32r baseline was PE-bound at 93%: every f32r matmul
# self-loads its stationary operand (f32r cannot use standalone LDWEIGHTS,
# serializing a ~150ns weight load with each stream) and 24 PE transposes
# ran per 128-edge tile.  Here the PE runs ONLY the 48 unavoidable N=512
# bf16 matmul streams per 128-edge tile (standalone LDWEIGHTS + FWL + the
# PE's pull-ahead hide the weight loads):
#
#   - edges are processed in supertiles of 512 (4 tiles of 128).
#   - gather: 4+4 indirect DMAs of 128 node rows each, f32 (exact).
#   - cast f32->bf16 on the vector engine.
#   - transpose: plain DMA of the bf16 rows to a DRAM scratch tile, then a
#     DMA-XBAR transpose DRAM->SBUF (the only direction the XBAR supports
#     on HW: SBUF->SBUF XBAR and the swdge sbuf transpose-gather both
#     return corrupt data) produces featT [128, 8, 512] (feature dim on
#     partitions, edges on free).  The src chain runs on the sync HWDGE
#     queue, the dst chain on the scalar HWDGE queue, so the two bounce+
#     transpose chains never head-block each other.
#   - layer 1 computed TRANSPOSED: psum[hid_m, 512e] += W1_blk[k,m]^T @
#     featT_k (stationary = W1 block, moving = featT, N=512).  Layer 1
#     output is then already h^T, so NO h transpose exists; bias+relu is
#     fused into one scalar-engine op per chunk (bias is per-partition in
#     this layout), writing hT [128, 8, 512] bf16 straight to SBUF.
#   - layer 2: stationary = hT edge-slices, moving = W2 (N=512), psum
#     [128e, 1024] -> +b2 on DVE -> store via an AP that undoes the
#     supertile edge interleave.
#   - prefetch chains are issued ~2 supertiles (~80us of PE work) ahead
#     and LAST within each iteration, so compute-side engine queues are
#     never head-blocked by prefetch waits; layer 2 trails layer 1 by one
#     supertile.
import os
import sys

import numpy as np

for _p in ("/opt/trn_rl_repo",):
    if _p not in sys.path:
        sys.path.insert(0, _p)

N_NODES = 50000
D_NODE = 1024
D_HID = 1024
N_CORES = 8
E_TOTAL = 65536
E_CORE = E_TOTAL // N_CORES
P = 128
SUP = 4  # tiles of 128 edges per supertile


def build_nc(e_core=E_CORE, n_nodes=N_NODES):
    import concourse.bass as bass
    import concourse.mybir as mybir
    import concourse.tile as tile
    from concourse import bacc

    f32 = mybir.dt.float32
    bf16 = mybir.dt.bfloat16
    i32 = mybir.dt.int32
    i16 = mybir.dt.int16

    T = e_core // P  # 64 edge tiles per core
    S = T // SUP  # 16 supertiles per core
    ES = SUP * P  # 512 edges per supertile
    KD = (2 * D_NODE) // P  # 16 contraction chunks, layer 1
    KH = D_HID // P  # 8 contraction chunks, layer 2
    NH = 512  # matmul moving free dim (one PSUM bank of fp32)

    nc = bacc.Bacc(None, target_bir_lowering=False)

    nf = nc.dram_tensor("node_feats", [n_nodes, D_NODE], f32, kind="ExternalInput")
    w1 = nc.dram_tensor("W1", [2 * D_NODE, D_HID], f32, kind="ExternalInput")
    w2 = nc.dram_tensor("W2", [D_HID, D_HID], f32, kind="ExternalInput")
    b1 = nc.dram_tensor("b1", [D_HID], f32, kind="ExternalInput")
    b2 = nc.dram_tensor("b2", [D_HID], f32, kind="ExternalInput")
    src = nc.dram_tensor("src", [e_core], i32, kind="ExternalInput")
    dst = nc.dram_tensor("dst", [e_core], i32, kind="ExternalInput")
    out = nc.dram_tensor("out", [e_core, D_HID], f32, kind="ExternalOutput")

    nf_ap = nf.ap()
    # edge e = p*T + s*SUP + j -> supertile s, scratch row r = p*SUP + j.
    # The store for edge-group g of supertile s covers rows {p in
    # [32g, 32g+32), j in [0, 4)}; iterating (p, j, h) row-major matches the
    # SBUF tile's partition order local = 4*(p - 32g) + j.
    out_r = out.ap().rearrange("(p s j) h -> s p j h", s=S, j=SUP)

    with tile.TileContext(nc) as tc:
        with (
            tc.tile_pool(name="const", bufs=1) as const_pool,
            tc.tile_pool(name="wpool", bufs=1) as wpool,
            tc.tile_pool(name="gather", bufs=2) as gather_pool,
            tc.tile_pool(name="gbf", bufs=2) as gbf_pool,
            tc.tile_pool(name="featT", bufs=2) as featT_pool,
            tc.tile_pool(name="hT", bufs=2) as hT_pool,
            tc.tile_pool(name="outp", bufs=2) as out_pool,
            tc.tile_pool(name="scr", bufs=2, space="DRAM") as scr_pool,
            tc.tile_pool(name="psT", bufs=2, space="PSUM") as psT_pool,
            tc.tile_pool(name="ps2", bufs=2, space="PSUM") as ps2_pool,
        ):
            idx_src = const_pool.tile([P, T], i32)
            idx_dst = const_pool.tile([P, T], i32)
            nc.sync.dma_start(idx_src[:], src.ap().rearrange("(p t) -> p t", t=T))
            nc.sync.dma_start(idx_dst[:], dst.ap().rearrange("(p t) -> p t", t=T))

            # weight tiles are declared here but loaded after the first
            # prefetch stage so supertile 0's gathers go first.
            w1_sb = wpool.tile([P, KD, D_HID], bf16)
            w1_v = w1.ap().rearrange("(k p) h -> p k h", p=P)
            w2_sb = wpool.tile([P, KH, D_HID], bf16)
            w2_v = w2.ap().rearrange("(k p) h -> p k h", p=P)

            def load_weights():
                # Casting loads go through the software DGE (gpsimd).
                # Interleave W1/W2 chunk loads 2:1 so layer-2 weights arrive
                # early.
                for k in range(KH):
                    nc.gpsimd.dma_start(w1_sb[:, 2 * k], w1_v[:, 2 * k])
                    nc.gpsimd.dma_start(w1_sb[:, 2 * k + 1], w1_v[:, 2 * k + 1])
                    nc.gpsimd.dma_start(w2_sb[:, k], w2_v[:, k])

            # b1 per-partition: b1_pp[p, m] = b1[m*128 + p]
            b1_pp = const_pool.tile([P, KH], f32)
            nc.sync.dma_start(b1_pp[:], b1.ap().rearrange("(m p) -> p m", p=P))
            # b2 broadcast to all partitions
            b2_bc = const_pool.tile([P, D_HID], f32)
            nc.sync.dma_start(b2_bc[:], b2.ap()[None, :].to_broadcast([P, D_HID]))

            def stage_P(s):
                """Prefetch: gathers -> DVE cast -> DRAM bounce -> XBAR.

                All prefetch DMAs live on the sync queue and the casts on the
                vector queue: the scalar queue must stay reserved for the
                relus, which free the layer-1 PSUM buffers — any DMA wait
                queued ahead of them stalls the PE."""
                fTs = []
                for name, idx in (("s", idx_src), ("d", idx_dst)):
                    g_f = gather_pool.tile([P, SUP, D_NODE], f32, tag="g" + name)
                    for j in range(SUP):
                        nc.gpsimd.indirect_dma_start(
                            out=g_f[:, j],
                            out_offset=None,
                            in_=nf_ap[:],
                            in_offset=bass.IndirectOffsetOnAxis(
                                ap=idx[:, SUP * s + j : SUP * s + j + 1], axis=0
                            ),
                        )
                    g_bf = gbf_pool.tile([P, SUP, D_NODE], bf16, tag="b" + name)
                    nc.vector.tensor_copy(g_bf[:], g_f[:])
                    scr = scr_pool.tile([ES, D_NODE], bf16, tag="scr" + name)
                    nc.sync.dma_start(
                        scr[:].rearrange("(p j) h -> p j h", j=SUP), g_bf[:]
                    )
                    fT = featT_pool.tile([P, KH, ES], bf16, tag="fT" + name)
                    nc.sync.dma_start(fT[:], scr[:], transpose=True)
                    fTs.append(fT)
                return fTs

            def stage_L1(s, fTs):
                """Layer 1, transposed: psum[hid_m, 512e]; fused bias+relu."""
                fT_s, fT_d = fTs
                hT = hT_pool.tile([P, KH, ES], bf16, tag="hT")
                for pair in range(KH // 2):
                    ps = psT_pool.tile([P, 2 * NH], f32, tag="psT")
                    for half in range(2):
                        m = 2 * pair + half
                        for k in range(KD):
                            fT = fT_s[:, k, :] if k < KH else fT_d[:, k - KH, :]
                            nc.tensor.matmul(
                                ps[:, half * NH : (half + 1) * NH],
                                w1_sb[:, k, m * P : (m + 1) * P],
                                fT,
                                start=(k == 0),
                                stop=(k == KD - 1),
                            )
                    for half in range(2):
                        m = 2 * pair + half
                        nc.scalar.activation(
                            hT[:, m, :],
                            ps[:, half * NH : (half + 1) * NH],
                            mybir.ActivationFunctionType.Relu,
                            bias=b1_pp[:, m : m + 1],
                        )
                return hT

            def stage_L2(s, hT):
                """Layer 2 per 128-edge group, +b2, interleaved store."""
                q = P // SUP
                for g in range(SUP):
                    ps2 = ps2_pool.tile([P, D_HID], f32, tag="ps2")
                    for half in range(2):
                        for k in range(KH):
                            nc.tensor.matmul(
                                ps2[:, half * NH : (half + 1) * NH],
                                hT[:, k, g * P : (g + 1) * P],
                                w2_sb[:, k, half * NH : (half + 1) * NH],
                                start=(k == 0),
                                stop=(k == KH - 1),
                            )
                    o_sb = out_pool.tile([P, D_HID], f32, tag="osb")
                    nc.vector.tensor_add(o_sb[:], ps2[:], b2_bc[:])
                    nc.sync.dma_start(out_r[s, g * q : (g + 1) * q], o_sb[:])

            # software pipeline: prefetch ~2 supertiles ahead (supertile 0's
            # gathers precede the 12.6MB weight load); L2 trails L1 by one
            # supertile so layer-2 matmuls never wait on the relu chain.
            # Prefetch is issued 3 supertiles ahead: the featT buffer ring
            # (bufs=2) naturally holds the final XBAR back until its slot
            # frees, but the gathers/casts/bounces of s+3 run a full
            # iteration earlier, which cuts the pipeline-ramp stalls.
            fTs = {0: stage_P(0)}
            load_weights()
            for ss in range(1, min(3, S)):
                fTs[ss] = stage_P(ss)
            hTs = {0: stage_L1(0, fTs.pop(0))}
            for s in range(S):
                if s + 1 < S:
                    hTs[s + 1] = stage_L1(s + 1, fTs.pop(s + 1))
                if s + 3 < S:
                    fTs[s + 3] = stage_P(s + 3)
                stage_L2(s, hTs.pop(s))

    nc.compile()
    return nc


LAST_RESULTS = None


def kernel(**inputs):
    global LAST_RESULTS
    from concourse.bass_utils import run_bass_kernel_spmd

    node_feats = np.ascontiguousarray(np.asarray(inputs["node_feats"], np.float32))
    W1 = np.ascontiguousarray(np.asarray(inputs["W1"], np.float32))
    W2 = np.ascontiguousarray(np.asarray(inputs["W2"], np.float32))
    b1 = np.ascontiguousarray(np.asarray(inputs["b1"], np.float32))
    b2 = np.ascontiguousarray(np.asarray(inputs["b2"], np.float32))
    src = np.ascontiguousarray(np.asarray(inputs["src"]).astype(np.int32))
    dst = np.ascontiguousarray(np.asarray(inputs["dst"]).astype(np.int32))

    nc = build_nc()

    in_maps = []
    for c in range(N_CORES):
        sl = slice(c * E_CORE, (c + 1) * E_CORE)
        in_maps.append(
            {
                "node_feats": node_feats,
                "W1": W1,
                "W2": W2,
                "b1": b1,
                "b2": b2,
                "src": src[sl],
                "dst": dst[sl],
            }
        )

    trace = bool(int(os.environ.get("KERNEL_TRACE", "0")))
    kw = {}
    if trace and bool(int(os.environ.get("KERNEL_TRACE_ALL", "0"))):
        kw["trace_cores"] = list(range(N_CORES))
    res = run_bass_kernel_spmd(
        nc, in_maps, core_ids=list(range(N_CORES)), trace=trace, **kw
    )
    LAST_RESULTS = res
    return np.concatenate([r["out"] for r in res.results], axis=0)
